# revision 13
# baseline (speedup 1.0000x reference)
"""Trainium2 Bass kernel for nn_LocalDownsample (segment mean-pool via one-hot matmul).

Contract: kernel(**inputs) takes FULL inputs (x [8,4096,512] f32,
regions [8,4096] i64, max_n=512), returns FULL output [8,512,512] f32.

Sharding: pure data parallel - batch b -> core b. Per core:
  out[n-1, :] = mean over tokens t with regions[t] == n of x[t, :]   (0 if empty)

Device algorithm per core (T=4096 tokens, C=512 channels, N=512 regions),
mode "fp8" (default):
  tokens laid out as t = j*128 + p (j = k-tile, p = SBUF partition).
  Host pre-quantizes x into one combined stream: row = [e4m3(x) | 1.0 | pad |
  e4m3(x - hi)] (1040 B). The constant 1.0 column makes segment *counts* fall
  out of the same matmuls that compute segment sums, already in partition
  layout (no transpose, no separate count pass); the hi+lo split recovers
  ~1e-3 relative error (vs 3.2e-2 for single fp8) at almost no PE cost
  because the extra matmuls reuse the already-loaded stationary one-hot.
  DVE builds the one-hot tiles directly in fp8 at 16-bit 4x speed: each
  uint16 halfword covers a PAIR of fp8 region columns, written as
  (iotaU == (r-1)>>1) * (0x0038 or 0x3800) - the byte pattern of fp8 1.0
  in the low or high half.
  PE runs fp8 DoubleRow matmuls (2 k-tiles = 256 tokens per instruction):
  per k-tile pair q and output row chunk m, four accumulating matmuls
  (hi/lo x two column halves) share one stationary load - a post-legalize
  pass drops the redundant LDWEIGHTS reloads (Tile emits one per matmul).
  acc[m] = [sum | counts] in PSUM (8 banks exactly). Final per m:
  rt = 1/max(cnt,1) on DVE, scale on ACT, 256 KiB DMA out, pipelined
  against the remaining m's matmuls (the last 8 pairs run m-major).

mode "fp16": same structure, single fp16 x stream (row [512 x | 1.0 | pad]),
  plain fp16 matmuls per k-tile; rel err ~2.5e-4, ~2x the PE work.
"""

import numpy as np
import ml_dtypes

import concourse.bacc as bacc
import concourse.bass as bass  # noqa: F401
import concourse.mybir as mybir
import concourse.tile as tile
from concourse.bass_utils import run_bass_kernel_spmd

P = 128          # SBUF partitions
T = 4096         # tokens per batch
C = 512          # channels
NR = 512         # number of regions (max_n)
JT = T // P      # 32 k-tiles
NPAIR = JT // 2  # 16 k-tile pairs (fp8 DoubleRow)
MC = NR // P     # 4 output row chunks
NCORES = 8
ROWB = 1040      # fp8 row bytes: 512 hi + 1 one + 15 pad + 512 lo
LO0 = 528        # byte offset of the lo half within a row
HIW = C + 8      # fp16 row elems: 512 x + 1 one + 7 pad (1040 B)
CHUNKS = (2, 2, 4, 8, 16)  # k-tiles per x DMA chunk (ramped start; even)

F8 = mybir.dt.float8e4
F16 = mybir.dt.float16
F32 = mybir.dt.float32
I32 = mybir.dt.int32
U16 = mybir.dt.uint16
NP_F8 = ml_dtypes.float8_e4m3

DEFAULT_CFG = dict(mode="fp8", dedup_ldw=True, rdma="gpsimd", repeats=1)

_CACHE = {}

# ---------------------------------------------------------------------------
# Post-legalize LDWEIGHTS dedup: Tile's legalizer splits every InstMatmult
# into InstLdweights + non-self-loading InstMatmult. Consecutive matmuls in
# a (pair, m) group share the same stationary one-hot block, so all but the
# first reload are redundant; drop them before semaphore assignment.
_DEDUP_ACTIVE = False


def _ldw_sig(inst):
    try:
        w = inst.ins[0]
        return (repr(w), repr(inst.perf_mode), repr(inst.tile_position),
                repr(inst.is_transpose))
    except Exception:
        return None


def _install_dedup():
    import concourse.tile as _tile
    if getattr(_tile.tile_legalize, "_ldw_dedup", False):
        return
    orig = _tile.tile_legalize

    def wrapped(ordered, nc):
        out = orig(ordered, nc)
        if not _DEDUP_ACTIVE:
            return out
        ndrop = 0
        for bb in list(out.keys()):
            insts = out[bb]
            new = []
            last_sig = None
            for inst in insts:
                if isinstance(inst, mybir.InstLdweights):
                    sig = _ldw_sig(inst)
                    if sig is not None and sig == last_sig:
                        ndrop += 1
                        continue
                    last_sig = sig
                elif isinstance(inst, mybir.InstMatmult):
                    if inst.ldweights not in (False, None):
                        last_sig = None
                elif getattr(inst, "engine", None) == mybir.EngineType.PE:
                    last_sig = None
                new.append(inst)
            out[bb] = new
        return out

    wrapped._ldw_dedup = True
    _tile.tile_legalize = wrapped


def _build(mode="fp8", dedup_ldw=True, rdma="gpsimd", repeats=1):
    global _DEDUP_ACTIVE
    assert sum(CHUNKS) == JT and all(c % 2 == 0 for c in CHUNKS)
    fp8 = mode in ("fp8", "fp8sw")
    swil = mode == "fp8sw"
    _install_dedup()
    _DEDUP_ACTIVE = bool(dedup_ldw)
    nc = bacc.Bacc(None, target_bir_lowering=False)
    if fp8:
        x_d = nc.dram_tensor("xc", [T, ROWB], F8, kind="ExternalInput")
    else:
        x_d = nc.dram_tensor("xc", [T, HIW], F16, kind="ExternalInput")
    r_d = nc.dram_tensor("regions", [P, JT], I32, kind="ExternalInput")
    o_d = nc.dram_tensor("out", [NR, C], F32, kind="ExternalOutput")

    DR = (mybir.MatmulPerfMode.DoubleRowSwInterleave if swil
          else mybir.MatmulPerfMode.DoubleRow)

    try:
        with tile.TileContext(nc) as tc:
            with (
                tc.tile_pool(name="const", bufs=1) as cpool,
                tc.tile_pool(name="xc", bufs=len(CHUNKS)) as x_pool,
                tc.tile_pool(name="oh", bufs=1) as oh_pool,
                tc.tile_pool(name="eplg", bufs=1) as out_pool,
                tc.tile_pool(name="psum", bufs=1, space="PSUM") as psum_pool,
            ):
                # --- constants + per-partition scalars (regions ride the
                # idle gpsimd SWDGE ring; the SP ring belongs to x from
                # t=0 and the ACT ring's start is taken by the act-table
                # load for the epilogue scale) ---
                r_i = cpool.tile([P, JT], I32, tag="r_i")
                r_eng = {"gpsimd": nc.gpsimd, "sync": nc.sync,
                         "scalar": nc.scalar}[rdma]
                r_eng.dma_start(r_i[:], r_d[:])

                if swil:
                    # descending-per-block region iota for the interleaved
                    # weight layout: riota[g] = 128*(g>>7) + 127 - (g&127)
                    riota = cpool.tile([P, NR], U16, tag="riota")
                    nc.gpsimd.iota(riota[:], pattern=[[128, MC], [-1, P]],
                                   base=P - 1, channel_multiplier=0)
                    r1f = cpool.tile([P, JT], F32, tag="r1f")
                    nc.vector.tensor_scalar(
                        out=r1f[:], in0=r_i[:], scalar1=1, scalar2=None,
                        op0=mybir.AluOpType.subtract,
                    )
                elif fp8:
                    # halfword-pair index (r-1)>>1 and packed fp8-1.0 byte
                    # pattern 56 or 14336 depending on (r-1)&1
                    iotaU = cpool.tile([P, 256], U16, tag="iotaU")
                    nc.gpsimd.iota(iotaU[:], pattern=[[1, 256]], base=0,
                                   channel_multiplier=0)
                    r1_i = cpool.tile([P, JT], I32, tag="r1_i")
                    nc.vector.tensor_scalar(
                        out=r1_i[:], in0=r_i[:], scalar1=1, scalar2=None,
                        op0=mybir.AluOpType.subtract,
                    )
                    ch_i = cpool.tile([P, JT], I32, tag="ch_i")
                    nc.vector.tensor_scalar(
                        out=ch_i[:], in0=r1_i[:], scalar1=1, scalar2=None,
                        op0=mybir.AluOpType.logical_shift_right,
                    )
                    od_i = cpool.tile([P, JT], I32, tag="od_i")
                    nc.vector.tensor_scalar(
                        out=od_i[:], in0=r1_i[:], scalar1=1, scalar2=None,
                        op0=mybir.AluOpType.bitwise_and,
                    )
                    val_i = cpool.tile([P, JT], I32, tag="val_i")
                    nc.vector.tensor_scalar(
                        out=val_i[:], in0=od_i[:], scalar1=14280, scalar2=56,
                        op0=mybir.AluOpType.mult,
                        op1=mybir.AluOpType.add,
                    )
                    # scalar operands must be float32 (values <= 14336, exact)
                    ch_u = cpool.tile([P, JT], F32, tag="ch_u")
                    nc.vector.tensor_copy(ch_u[:], ch_i[:])
                    val_u = cpool.tile([P, JT], F32, tag="val_u")
                    nc.vector.tensor_copy(val_u[:], val_i[:])
                else:
                    iota16 = cpool.tile([P, NR], F16, tag="iota16")
                    nc.gpsimd.iota(
                        iota16[:], pattern=[[1, NR]], base=1,
                        channel_multiplier=0,
                        allow_small_or_imprecise_dtypes=True,  # 1..512 exact
                    )
                    r_f = cpool.tile([P, JT], F32, tag="r_f")
                    nc.vector.tensor_copy(r_f[:], r_i[:])

                def body():
                    xv = x_d.rearrange("(j p) b -> p j b", p=P)
                    xt = []
                    j0 = 0
                    for ci, csz in enumerate(CHUNKS):
                        t = x_pool.tile([P, csz, ROWB if fp8 else HIW],
                                        F8 if fp8 else F16,
                                        name=f"xc{ci}", tag="xc")
                        nc.sync.dma_start(t[:], xv[:, j0:j0 + csz, :])
                        for k in range(csz):
                            xt.append((t, k))
                        j0 += csz

                    # --- one-hot tiles (DVE, all 32 up-front; they only
                    # depend on the tiny regions DMA + prep) ---
                    if swil:
                        # interleaved-reversed weight layout per pair:
                        # bytes [A_127, B_127, A_126, B_126, ..., B_0] per
                        # m-block; as u16: 56*[riota==rA] + 14336*[riota==rB]
                        oh = oh_pool.tile([P, NPAIR, 2 * NR], F8, tag="oh")
                        ohu = oh.bitcast(U16)  # [P, NPAIR, NR]
                        tmp = oh_pool.tile([P, NR], U16, tag="ohtmp")
                        for q in range(NPAIR):
                            nc.vector.tensor_scalar(
                                out=tmp[:], in0=riota[:],
                                scalar1=r1f[:, 2 * q + 1:2 * q + 2],
                                scalar2=14336.0,
                                op0=mybir.AluOpType.is_equal,
                                op1=mybir.AluOpType.mult,
                            )
                            nc.vector.tensor_scalar(
                                out=ohu[:, q, :], in0=riota[:],
                                scalar1=r1f[:, 2 * q:2 * q + 1],
                                scalar2=56.0,
                                op0=mybir.AluOpType.is_equal,
                                op1=mybir.AluOpType.mult,
                            )
                            nc.vector.tensor_tensor(
                                out=ohu[:, q, :], in0=ohu[:, q, :],
                                in1=tmp[:], op=mybir.AluOpType.add,
                            )
                    elif fp8:
                        oh = oh_pool.tile([P, JT, NR], F8, tag="oh")
                        ohu = oh.bitcast(U16)  # [P, JT, 256]
                        for j in range(JT):
                            nc.vector.tensor_scalar(
                                out=ohu[:, j, :], in0=iotaU[:],
                                scalar1=ch_u[:, j:j + 1],
                                scalar2=val_u[:, j:j + 1],
                                op0=mybir.AluOpType.is_equal,
                                op1=mybir.AluOpType.mult,
                            )
                    else:
                        oh = oh_pool.tile([P, JT, NR], F16, tag="oh")
                        for j in range(JT):
                            nc.vector.tensor_scalar(
                                out=oh[:, j, :], in0=iota16[:],
                                scalar1=r_f[:, j:j + 1], scalar2=None,
                                op0=mybir.AluOpType.is_equal,
                            )

                    # --- PSUM: one full bank per (m, half): A = x cols
                    # 0:256, B = x cols 256:512 + count col at B[:, 256] ---
                    accA = [
                        psum_pool.tile([P, C], F32, name=f"accA{m}",
                                       tag=f"accA{m}")
                        for m in range(MC)
                    ]
                    accB = [
                        psum_pool.tile([P, C], F32, name=f"accB{m}",
                                       tag=f"accB{m}")
                        for m in range(MC)
                    ]

                    def mm(out_ap, w, rhs, start, stop):
                        nc.tensor.matmul(
                            out_ap, lhsT=w, rhs=rhs, start=start, stop=stop,
                            perf_mode=DR if fp8 else None,
                            skip_group_check=True,
                        )

                    def group(q, m, startq, stopq):
                        # matmuls of one (q, m) share the stationary one-hot
                        if fp8:
                            t, k = xt[2 * q]
                            if swil:
                                w = oh[:, q, 2 * P * m:2 * P * (m + 1)]
                            else:
                                w = oh[:, 2 * q:2 * q + 2, m * P:(m + 1) * P]
                            mm(accA[m][:, 0:256], w, t[:, k:k + 2, 0:256],
                               startq, False)
                            mm(accB[m][:, 0:257], w, t[:, k:k + 2, 256:513],
                               startq, False)
                            mm(accA[m][:, 0:256], w,
                               t[:, k:k + 2, LO0:LO0 + 256], False, stopq)
                            mm(accB[m][:, 0:256], w,
                               t[:, k:k + 2, LO0 + 256:LO0 + 512],
                               False, stopq)
                        else:
                            t, k = xt[q]
                            w = oh[:, q, m * P:(m + 1) * P]
                            mm(accA[m][:, 0:256], w, t[:, k, 0:256],
                               startq, stopq)
                            mm(accB[m][:, 0:257], w, t[:, k, 256:513],
                               startq, stopq)

                    NQ = NPAIR if fp8 else JT
                    LASTQ = NQ - CHUNKS[-1] // (2 if fp8 else 1)
                    for q in range(LASTQ):
                        for m in range(MC):
                            group(q, m, startq=(q == 0), stopq=False)

                    rt = out_pool.tile([P, MC], F32, tag="rt")
                    csb = out_pool.tile([P, MC], F32, tag="csb")
                    osb = out_pool.tile([P, MC, C], F32, tag="osb")
                    for m in range(MC):
                        for q in range(LASTQ, NQ):
                            group(q, m, startq=(q == 0 and LASTQ == 0),
                                  stopq=(q == NQ - 1))
                        # --- close m (overlaps later m's matmuls);
                        # out DMAs alternate HWDGE rings ---
                        nc.vector.tensor_scalar_max(
                            csb[:, m:m + 1], accB[m][:, 256:257], 1.0)
                        nc.vector.reciprocal(rt[:, m:m + 1], csb[:, m:m + 1])
                        nc.scalar.activation(
                            out=osb[:, m, 0:256], in_=accA[m][:, 0:256],
                            func=mybir.ActivationFunctionType.Copy,
                            scale=rt[:, m:m + 1],
                        )
                        nc.scalar.activation(
                            out=osb[:, m, 256:512], in_=accB[m][:, 0:256],
                            func=mybir.ActivationFunctionType.Copy,
                            scale=rt[:, m:m + 1],
                        )
                        ring = nc.sync if m % 2 == 0 else nc.scalar
                        ring.dma_start(o_d[m * P:(m + 1) * P, :], osb[:, m, :])

                if repeats == 1:
                    body()
                else:
                    with tc.For_i(0, repeats, 1,
                                  hint_engines=(mybir.EngineType.PE,)):
                        body()

        nc.compile()
    finally:
        _DEDUP_ACTIVE = False
    return nc


def _get_nc(**cfg):
    cfg = {**DEFAULT_CFG, **cfg}
    key = tuple(sorted(cfg.items()))
    if key not in _CACHE:
        _CACHE[key] = _build(**cfg)
    return _CACHE[key]


def _prepare_in_maps(x, regions, mode="fp8"):
    """Host-side shard + layout prep: per-core quantized x stream (with the
    constant 1.0 count column appended) and k-tile-major regions."""
    x = np.asarray(x, dtype=np.float32)
    r32 = np.asarray(regions).astype(np.int32)
    in_maps = []
    for b in range(NCORES):
        rt = np.ascontiguousarray(r32[b].reshape(JT, P).T)  # [P, JT]
        if mode in ("fp8", "fp8sw"):
            hi = x[b].astype(NP_F8)
            lo = (x[b] - hi.astype(np.float32)).astype(NP_F8)
            buf = np.zeros((T, ROWB), dtype=NP_F8)
            buf[:, :C] = hi
            buf[:, C] = NP_F8(1.0)
            buf[:, LO0:LO0 + C] = lo
        else:
            buf = np.zeros((T, HIW), dtype=np.float16)
            buf[:, :C] = x[b].astype(np.float16)
            buf[:, C] = np.float16(1.0)
        in_maps.append({"xc": buf, "regions": rt})
    return in_maps


def kernel(x, regions, max_n, _trace=False, _tmpdir=None, _cfg=None):
    x = np.asarray(x, dtype=np.float32)
    regions = np.asarray(regions)
    assert x.shape == (NCORES, T, C), x.shape
    assert regions.shape == (NCORES, T), regions.shape
    assert int(np.asarray(max_n)) == NR

    cfg = {**DEFAULT_CFG, **(_cfg or {})}
    nc = _get_nc(**cfg)
    in_maps = _prepare_in_maps(x, regions, mode=cfg["mode"])
    try:
        res = run_bass_kernel_spmd(
            nc,
            in_maps,
            core_ids=list(range(NCORES)),
            trace=_trace,
            tmpdir=_tmpdir,
        )
    except Exception:
        # one retry for transient runtime/tunnel failures
        res = run_bass_kernel_spmd(
            nc,
            in_maps,
            core_ids=list(range(NCORES)),
            trace=_trace,
            tmpdir=_tmpdir,
        )
    out = np.stack([res.results[b]["out"] for b in range(NCORES)], axis=0)
    if _trace:
        kernel._last_results = res
    return out


# revision 19
# speedup vs baseline: 1.1730x; 1.1730x over previous
"""Trainium2 Bass kernel for nn_LocalDownsample (segment mean-pool via one-hot matmul).

Contract: kernel(**inputs) takes FULL inputs (x [8,4096,512] f32,
regions [8,4096] i64, max_n=512), returns FULL output [8,512,512] f32.

Sharding: pure data parallel - batch b -> core b. Per core:
  out[n-1, :] = mean over tokens t with regions[t] == n of x[t, :]   (0 if empty)

Device algorithm per core (T=4096 tokens, C=512 channels, N=512 regions),
mode "fp8" (default):
  tokens laid out as t = j*128 + p (j = k-tile, p = SBUF partition).
  Host pre-quantizes x into one combined stream: row = [e4m3(x) | 1.0 | pad |
  e4m3(x - hi)] (1040 B). The constant 1.0 column makes segment *counts* fall
  out of the same matmuls that compute segment sums, already in partition
  layout (no transpose, no separate count pass); the hi+lo split recovers
  ~1e-3 relative error (vs 3.2e-2 for single fp8) at almost no PE cost
  because the extra matmuls reuse the already-loaded stationary one-hot.
  DVE builds the one-hot tiles directly in fp8 at 16-bit 4x speed: each
  uint16 halfword covers a PAIR of fp8 region columns, written as
  (iotaU == (r-1)>>1) * (0x0038 or 0x3800) - the byte pattern of fp8 1.0
  in the low or high half.
  PE runs fp8 DoubleRow matmuls (2 k-tiles = 256 tokens per instruction):
  per k-tile pair q and output row chunk m, four accumulating matmuls
  (hi/lo x two column halves) share one stationary load - a post-legalize
  pass drops the redundant LDWEIGHTS reloads (Tile emits one per matmul).
  acc[m] = [sum | counts] in PSUM (8 banks exactly). Final per m:
  rt = 1/max(cnt,1) on DVE, scale on ACT, 256 KiB DMA out, pipelined
  against the remaining m's matmuls (the last 8 pairs run m-major).

mode "fp16": same structure, single fp16 x stream (row [512 x | 1.0 | pad]),
  plain fp16 matmuls per k-tile; rel err ~2.5e-4, ~2x the PE work.
"""

import numpy as np
import ml_dtypes

import concourse.bacc as bacc
import concourse.bass as bass  # noqa: F401
import concourse.mybir as mybir
import concourse.tile as tile
from concourse.bass_utils import run_bass_kernel_spmd

P = 128          # SBUF partitions
T = 4096         # tokens per batch
C = 512          # channels
NR = 512         # number of regions (max_n)
JT = T // P      # 32 k-tiles
NPAIR = JT // 2  # 16 k-tile pairs (fp8 DoubleRow)
MC = NR // P     # 4 output row chunks
NCORES = 8
ROWB = 1040      # fp8 row bytes: 512 hi + 1 one + 15 pad + 512 lo
LO0 = 528        # byte offset of the lo half within a row
HIW = C + 8      # fp16 row elems: 512 x + 1 one + 7 pad (1040 B)
CHUNKS = (2, 2, 4, 8, 16)  # k-tiles per x DMA chunk (ramped start; even)

F8 = mybir.dt.float8e4
F16 = mybir.dt.float16
F32 = mybir.dt.float32
I32 = mybir.dt.int32
U16 = mybir.dt.uint16
NP_F8 = ml_dtypes.float8_e4m3

DEFAULT_CFG = dict(mode="fp8", dedup_ldw=True, rdma="gpsimd", stagger=True,
                   tailq=12, repeats=1)

_CACHE = {}

# ---------------------------------------------------------------------------
# Post-legalize LDWEIGHTS dedup: Tile's legalizer splits every InstMatmult
# into InstLdweights + non-self-loading InstMatmult. Consecutive matmuls in
# a (pair, m) group share the same stationary one-hot block, so all but the
# first reload are redundant; drop them before semaphore assignment.
_DEDUP_ACTIVE = False


def _ldw_sig(inst):
    try:
        w = inst.ins[0]
        return (repr(w), repr(inst.perf_mode), repr(inst.tile_position),
                repr(inst.is_transpose))
    except Exception:
        return None


def _install_dedup():
    import concourse.tile as _tile
    if getattr(_tile.tile_legalize, "_ldw_dedup", False):
        return
    orig = _tile.tile_legalize

    def wrapped(ordered, nc):
        out = orig(ordered, nc)
        if not _DEDUP_ACTIVE:
            return out
        ndrop = 0
        for bb in list(out.keys()):
            insts = out[bb]
            new = []
            last_sig = None
            for inst in insts:
                if isinstance(inst, mybir.InstLdweights):
                    sig = _ldw_sig(inst)
                    if sig is not None and sig == last_sig:
                        ndrop += 1
                        continue
                    last_sig = sig
                elif isinstance(inst, mybir.InstMatmult):
                    if inst.ldweights not in (False, None):
                        last_sig = None
                elif getattr(inst, "engine", None) == mybir.EngineType.PE:
                    last_sig = None
                new.append(inst)
            out[bb] = new
        return out

    wrapped._ldw_dedup = True
    _tile.tile_legalize = wrapped


def _build(mode="fp8", dedup_ldw=True, rdma="gpsimd", probe=None,
           stagger=False, wz=False, tailq=8, xbufs=0, repeats=1):
    global _DEDUP_ACTIVE
    chunks = (JT,) if probe == "onechunk" else CHUNKS
    assert sum(chunks) == JT and all(c % 2 == 0 for c in chunks)
    fp8 = mode in ("fp8", "fp8sw")
    swil = mode == "fp8sw"
    _install_dedup()
    _DEDUP_ACTIVE = bool(dedup_ldw)
    nc = bacc.Bacc(None, target_bir_lowering=False)
    if fp8:
        x_d = nc.dram_tensor("xc", [T, ROWB], F8, kind="ExternalInput")
    else:
        x_d = nc.dram_tensor("xc", [T, HIW], F16, kind="ExternalInput")
    r_d = nc.dram_tensor("regions", [P, JT], I32, kind="ExternalInput")
    o_d = nc.dram_tensor("out", [NR, C], F32, kind="ExternalOutput")

    DR = (mybir.MatmulPerfMode.DoubleRowSwInterleave if swil
          else mybir.MatmulPerfMode.DoubleRow)

    try:
        with tile.TileContext(nc) as tc:
            with (
                tc.tile_pool(name="const", bufs=1) as cpool,
                tc.tile_pool(name="xc", bufs=len(chunks) + xbufs) as x_pool,
                tc.tile_pool(name="oh", bufs=1) as oh_pool,
                tc.tile_pool(name="eplg", bufs=1) as out_pool,
                tc.tile_pool(name="psum", bufs=1, space="PSUM") as psum_pool,
            ):
                # --- constants + per-partition scalars (regions ride the
                # idle gpsimd SWDGE ring; the SP ring belongs to x from
                # t=0 and the ACT ring's start is taken by the act-table
                # load for the epilogue scale) ---
                r_i = cpool.tile([P, JT], I32, tag="r_i")
                r_eng = {"gpsimd": nc.gpsimd, "sync": nc.sync,
                         "scalar": nc.scalar}[rdma]
                r_eng.dma_start(r_i[:], r_d[:])

                if swil:
                    # descending-per-block region iota for the interleaved
                    # weight layout: riota[g] = 128*(g>>7) + 127 - (g&127)
                    riota = cpool.tile([P, NR], U16, tag="riota")
                    nc.gpsimd.iota(riota[:], pattern=[[128, MC], [-1, P]],
                                   base=P - 1, channel_multiplier=0)
                    r1f = cpool.tile([P, JT], F32, tag="r1f")
                    nc.vector.tensor_scalar(
                        out=r1f[:], in0=r_i[:], scalar1=1, scalar2=None,
                        op0=mybir.AluOpType.subtract,
                    )
                elif fp8:
                    # halfword-pair index (r-1)>>1 and packed fp8-1.0 byte
                    # pattern 56 or 14336 depending on (r-1)&1
                    iotaU = cpool.tile([P, 256], U16, tag="iotaU")
                    nc.gpsimd.iota(iotaU[:], pattern=[[1, 256]], base=0,
                                   channel_multiplier=0)
                    r1_i = cpool.tile([P, JT], I32, tag="r1_i")
                    nc.vector.tensor_scalar(
                        out=r1_i[:], in0=r_i[:], scalar1=1, scalar2=None,
                        op0=mybir.AluOpType.subtract,
                    )
                    ch_i = cpool.tile([P, JT], I32, tag="ch_i")
                    nc.vector.tensor_scalar(
                        out=ch_i[:], in0=r1_i[:], scalar1=1, scalar2=None,
                        op0=mybir.AluOpType.logical_shift_right,
                    )
                    od_i = cpool.tile([P, JT], I32, tag="od_i")
                    nc.vector.tensor_scalar(
                        out=od_i[:], in0=r1_i[:], scalar1=1, scalar2=None,
                        op0=mybir.AluOpType.bitwise_and,
                    )
                    val_i = cpool.tile([P, JT], I32, tag="val_i")
                    nc.vector.tensor_scalar(
                        out=val_i[:], in0=od_i[:], scalar1=14280, scalar2=56,
                        op0=mybir.AluOpType.mult,
                        op1=mybir.AluOpType.add,
                    )
                    # scalar operands must be float32 (values <= 14336, exact)
                    ch_u = cpool.tile([P, JT], F32, tag="ch_u")
                    nc.vector.tensor_copy(ch_u[:], ch_i[:])
                    val_u = cpool.tile([P, JT], F32, tag="val_u")
                    nc.vector.tensor_copy(val_u[:], val_i[:])
                else:
                    iota16 = cpool.tile([P, NR], F16, tag="iota16")
                    nc.gpsimd.iota(
                        iota16[:], pattern=[[1, NR]], base=1,
                        channel_multiplier=0,
                        allow_small_or_imprecise_dtypes=True,  # 1..512 exact
                    )
                    r_f = cpool.tile([P, JT], F32, tag="r_f")
                    nc.vector.tensor_copy(r_f[:], r_i[:])

                def body():
                    xv = x_d.rearrange("(j p) b -> p j b", p=P)
                    xt = []
                    j0 = 0
                    for ci, csz in enumerate(chunks):
                        t = x_pool.tile([P, csz, ROWB if fp8 else HIW],
                                        F8 if fp8 else F16,
                                        name=f"xc{ci}", tag="xc")
                        nc.sync.dma_start(t[:], xv[:, j0:j0 + csz, :])
                        for k in range(csz):
                            xt.append((t, k))
                        j0 += csz

                    # --- one-hot tiles (DVE, all 32 up-front; they only
                    # depend on the tiny regions DMA + prep) ---
                    if swil:
                        # interleaved-reversed weight layout per pair:
                        # bytes [A_127, B_127, A_126, B_126, ..., B_0] per
                        # m-block; as u16: 56*[riota==rA] + 14336*[riota==rB]
                        oh = oh_pool.tile([P, NPAIR, 2 * NR], F8, tag="oh")
                        ohu = oh.bitcast(U16)  # [P, NPAIR, NR]
                        tmp = oh_pool.tile([P, NR], U16, tag="ohtmp")
                        for q in range(NPAIR):
                            nc.vector.tensor_scalar(
                                out=tmp[:], in0=riota[:],
                                scalar1=r1f[:, 2 * q + 1:2 * q + 2],
                                scalar2=14336.0,
                                op0=mybir.AluOpType.is_equal,
                                op1=mybir.AluOpType.mult,
                            )
                            nc.vector.tensor_scalar(
                                out=ohu[:, q, :], in0=riota[:],
                                scalar1=r1f[:, 2 * q:2 * q + 1],
                                scalar2=56.0,
                                op0=mybir.AluOpType.is_equal,
                                op1=mybir.AluOpType.mult,
                            )
                            nc.vector.tensor_tensor(
                                out=ohu[:, q, :], in0=ohu[:, q, :],
                                in1=tmp[:], op=mybir.AluOpType.add,
                            )
                    elif fp8:
                        oh = oh_pool.tile([P, JT, NR], F8, tag="oh")
                        ohu = oh.bitcast(U16)  # [P, JT, 256]
                        for j in range(JT):
                            nc.vector.tensor_scalar(
                                out=ohu[:, j, :], in0=iotaU[:],
                                scalar1=ch_u[:, j:j + 1],
                                scalar2=val_u[:, j:j + 1],
                                op0=mybir.AluOpType.is_equal,
                                op1=mybir.AluOpType.mult,
                            )
                    else:
                        oh = oh_pool.tile([P, JT, NR], F16, tag="oh")
                        for j in range(JT):
                            nc.vector.tensor_scalar(
                                out=oh[:, j, :], in0=iota16[:],
                                scalar1=r_f[:, j:j + 1], scalar2=None,
                                op0=mybir.AluOpType.is_equal,
                            )

                    # --- PSUM: one full bank per (m, half): A = x cols
                    # 0:256, B = x cols 256:512 + count col at B[:, 256] ---
                    accA = [
                        psum_pool.tile([P, C], F32, name=f"accA{m}",
                                       tag=f"accA{m}")
                        for m in range(MC)
                    ]
                    accB = [
                        psum_pool.tile([P, C], F32, name=f"accB{m}",
                                       tag=f"accB{m}")
                        for m in range(MC)
                    ]

                    def mm(out_ap, w, rhs, start, stop):
                        inst = nc.tensor.matmul(
                            out_ap, lhsT=w, rhs=rhs, start=start, stop=stop,
                            perf_mode=DR if fp8 else None,
                            skip_group_check=True,
                        )
                        if wz:
                            inst.is_weight_onezero = True

                    def group(q, m, startq, stopq):
                        # matmuls of one (q, m) share the stationary one-hot
                        if fp8:
                            t, k = xt[2 * q]
                            if swil:
                                w = oh[:, q, 2 * P * m:2 * P * (m + 1)]
                            else:
                                w = oh[:, 2 * q:2 * q + 2, m * P:(m + 1) * P]
                            if probe == "nolo":
                                mm(accA[m][:, 0:256], w,
                                   t[:, k:k + 2, 0:256], startq, stopq)
                                mm(accB[m][:, 0:257], w,
                                   t[:, k:k + 2, 256:513], startq, stopq)
                            else:
                                mm(accA[m][:, 0:256], w,
                                   t[:, k:k + 2, 0:256], startq, False)
                                mm(accB[m][:, 0:257], w,
                                   t[:, k:k + 2, 256:513], startq, False)
                                mm(accA[m][:, 0:256], w,
                                   t[:, k:k + 2, LO0:LO0 + 256], False, stopq)
                                mm(accB[m][:, 0:256], w,
                                   t[:, k:k + 2, LO0 + 256:LO0 + 512],
                                   False, stopq)
                        else:
                            t, k = xt[q]
                            w = oh[:, q, m * P:(m + 1) * P]
                            mm(accA[m][:, 0:256], w, t[:, k, 0:256],
                               startq, stopq)
                            mm(accB[m][:, 0:257], w, t[:, k, 256:513],
                               startq, stopq)

                    NQ = NPAIR if fp8 else JT
                    LASTQ = NQ - (tailq if fp8 else 2 * tailq)
                    for q in range(LASTQ):
                        for m in range(MC):
                            group(q, m, startq=(q == 0), stopq=False)

                    rt = out_pool.tile([P, MC], F32, tag="rt")
                    csb = out_pool.tile([P, MC], F32, tag="csb")
                    osb = out_pool.tile([P, MC, C], F32, tag="osb")
                    for m in range(MC):
                        for q in range(LASTQ, NQ):
                            group(q, m, startq=(q == 0 and LASTQ == 0),
                                  stopq=(q == NQ - 1))
                        # --- close m (overlaps later m's matmuls);
                        # out DMAs alternate HWDGE rings ---
                        nc.vector.tensor_scalar_max(
                            csb[:, m:m + 1], accB[m][:, 256:257], 1.0)
                        nc.vector.reciprocal(rt[:, m:m + 1], csb[:, m:m + 1])
                        nc.scalar.activation(
                            out=osb[:, m, 0:256], in_=accA[m][:, 0:256],
                            func=mybir.ActivationFunctionType.Copy,
                            scale=rt[:, m:m + 1],
                        )
                        nc.vector.tensor_scalar(
                            out=osb[:, m, 256:512], in0=accB[m][:, 0:256],
                            scalar1=rt[:, m:m + 1], scalar2=None,
                            op0=mybir.AluOpType.mult,
                        )
                        ring0 = nc.sync if m % 2 == 0 else nc.scalar
                        ring1 = nc.scalar if m % 2 == 0 else nc.sync
                        ring0.dma_start(o_d[m * P:(m + 1) * P, 0:256],
                                        osb[:, m, 0:256])
                        ring1.dma_start(o_d[m * P:(m + 1) * P, 256:512],
                                        osb[:, m, 256:512])

                if repeats == 1:
                    body()
                else:
                    with tc.For_i(0, repeats, 1,
                                  hint_engines=(mybir.EngineType.PE,),
                                  staggered_reset=stagger):
                        body()

        nc.compile()
    finally:
        _DEDUP_ACTIVE = False
    return nc


def _get_nc(**cfg):
    cfg = {**DEFAULT_CFG, **cfg}
    key = tuple(sorted(cfg.items()))
    if key not in _CACHE:
        _CACHE[key] = _build(**cfg)
    return _CACHE[key]


def _prepare_in_maps(x, regions, mode="fp8"):
    """Host-side shard + layout prep: per-core quantized x stream (with the
    constant 1.0 count column appended) and k-tile-major regions."""
    x = np.asarray(x, dtype=np.float32)
    r32 = np.asarray(regions).astype(np.int32)
    in_maps = []
    for b in range(NCORES):
        rt = np.ascontiguousarray(r32[b].reshape(JT, P).T)  # [P, JT]
        if mode in ("fp8", "fp8sw"):
            hi = x[b].astype(NP_F8)
            lo = (x[b] - hi.astype(np.float32)).astype(NP_F8)
            buf = np.zeros((T, ROWB), dtype=NP_F8)
            buf[:, :C] = hi
            buf[:, C] = NP_F8(1.0)
            buf[:, LO0:LO0 + C] = lo
        else:
            buf = np.zeros((T, HIW), dtype=np.float16)
            buf[:, :C] = x[b].astype(np.float16)
            buf[:, C] = np.float16(1.0)
        in_maps.append({"xc": buf, "regions": rt})
    return in_maps


def kernel(x, regions, max_n, _trace=False, _tmpdir=None, _cfg=None):
    x = np.asarray(x, dtype=np.float32)
    regions = np.asarray(regions)
    assert x.shape == (NCORES, T, C), x.shape
    assert regions.shape == (NCORES, T), regions.shape
    assert int(np.asarray(max_n)) == NR

    cfg = {**DEFAULT_CFG, **(_cfg or {})}
    nc = _get_nc(**cfg)
    in_maps = _prepare_in_maps(x, regions, mode=cfg["mode"])
    try:
        res = run_bass_kernel_spmd(
            nc,
            in_maps,
            core_ids=list(range(NCORES)),
            trace=_trace,
            tmpdir=_tmpdir,
        )
    except Exception:
        # one retry for transient runtime/tunnel failures
        res = run_bass_kernel_spmd(
            nc,
            in_maps,
            core_ids=list(range(NCORES)),
            trace=_trace,
            tmpdir=_tmpdir,
        )
    out = np.stack([res.results[b]["out"] for b in range(NCORES)], axis=0)
    if _trace:
        kernel._last_results = res
    return out
